# revision 2
# baseline (speedup 1.0000x reference)
"""Cross-modal MHA v3 on 8 TRN2 cores (data-parallel over batch).

vs baseline: swapped-orientation PV (out[q,d], lhsT=e) halves PV row count;
exp split across Act (table exp) and DVE (1-instr Schraudolph -> bf16 bits);
PE transposes + batched Act copies build concatT; bf16 DMA output with
host-side bias add. Q/K path all-bf16 (fp8 fails the error budget: weight
noise couples directly to the output, no softmax washing).
"""

import numpy as np

HEADS = 16
DM = 1024
IMG = 512
DK = 64
LQ = 2048
LKV = 1024
B = 8
P = 128
IB = 512

ESCALE = 0.125
ACT_EXP_NUM = 18       # of every 32 exp double-tiles, this many go to Act

_cache = {}
TRACE = False
LAST_RESULT = None


def _build_nc():
    from contextlib import ExitStack

    import concourse.tile as tile
    from concourse import bacc, mybir
    from concourse.masks import make_identity

    dt = mybir.dt
    f32 = dt.float32
    bf16 = dt.bfloat16
    u16 = dt.uint16
    Alu = mybir.AluOpType
    Exp = mybir.ActivationFunctionType.Exp
    Id = mybir.ActivationFunctionType.Identity
    Copy = mybir.ActivationFunctionType.Copy

    n_ib = LQ // IB            # 4 query blocks
    n_jt = LKV // P            # 8 kv tiles
    HW = DK + 1                # 65

    LN2 = float(np.log(2.0))
    C1 = ESCALE * (2.0 ** 23) / LN2 / 65536.0
    KP = (127.0 * (2.0 ** 23) - 366000.0) / 65536.0

    nc = bacc.Bacc("TRN2", target_bir_lowering=False, debug=False)

    qT16 = nc.declare_dram_parameter("qT16", [P, 8, LQ], bf16, isOutput=False)
    kT16 = nc.declare_dram_parameter("kT16", [P, 4, LKV], bf16, isOutput=False)
    vT16 = nc.declare_dram_parameter("vT16", [P, 4, LKV], bf16, isOutput=False)
    Wq16 = nc.declare_dram_parameter("Wq16", [P, 8, DM], bf16, isOutput=False)
    Wk16 = nc.declare_dram_parameter("Wk16", [P, 4, DM], bf16, isOutput=False)
    Wv16 = nc.declare_dram_parameter("Wv16", [P, 4, DM], bf16, isOutput=False)
    Wo16 = nc.declare_dram_parameter("Wo16", [P, 8, DM], bf16, isOutput=False)
    bqp = nc.declare_dram_parameter("bqp", [P, 8], f32, isOutput=False)
    bkp = nc.declare_dram_parameter("bkp", [P, 8], f32, isOutput=False)
    bv16 = nc.declare_dram_parameter("bv16", [1, DM], bf16, isOutput=False)
    out16 = nc.declare_dram_parameter("out16", [LQ, DM], bf16, isOutput=True)

    with tile.TileContext(nc) as tc, ExitStack() as ctx:
        singles = ctx.enter_context(tc.tile_pool(name="singles", bufs=1))
        qTp = ctx.enter_context(tc.tile_pool(name="qTp", bufs=3))
        qhp = ctx.enter_context(tc.tile_pool(name="qhp", bufs=3))
        e_pool = ctx.enter_context(tc.tile_pool(name="e", bufs=18))
        an_pool = ctx.enter_context(tc.tile_pool(name="an", bufs=18))
        rec_pool = ctx.enter_context(tc.tile_pool(name="rec", bufs=2))
        cat_pool = ctx.enter_context(tc.tile_pool(name="cat", bufs=1))
        osb_pool = ctx.enter_context(tc.tile_pool(name="osb", bufs=4))

        # PSUM: sc 2x2 + pv 2 + tr 1 + psg 1 = 8 banks
        ps_sc = ctx.enter_context(tc.tile_pool(name="ps_sc", bufs=2, space="PSUM"))
        ps_pv = ctx.enter_context(tc.tile_pool(name="ps_pv", bufs=2, space="PSUM"))
        ps_tr = ctx.enter_context(tc.tile_pool(name="ps_tr", bufs=1, space="PSUM"))
        ps_g = ctx.enter_context(tc.tile_pool(name="ps_g", bufs=1, space="PSUM"))

        kT_sb = singles.tile([P, 4, LKV], bf16)
        vT_sb = singles.tile([P, 4, LKV], bf16)
        Wq_sb = singles.tile([P, 8, DM], bf16)
        Wk_sb = singles.tile([P, 4, DM], bf16)
        Wv_sb = singles.tile([P, 4, DM], bf16)
        Wo_sb = singles.tile([P, 8, DM], bf16)
        bq_sb = singles.tile([P, 8], f32)
        bk_sb = singles.tile([P, 8], f32)
        bv_sb = singles.tile([1, DM], bf16)
        ones1 = singles.tile([1, P], bf16)
        ident = singles.tile([P, P], bf16)

        kh16 = singles.tile([P, 8, LKV], bf16)           # [dmo%128, do, kv]
        vh16 = singles.tile([P, n_jt, HEADS, HW], bf16)  # [kv%128, jt, h, d+1]

        nc.sync.dma_start(Wk_sb, Wk16[:, :, :])
        nc.sync.dma_start(kT_sb, kT16[:, :, :])
        nc.sync.dma_start(bk_sb, bkp[:, :])
        nc.sync.dma_start(bq_sb, bqp[:, :])
        nc.sync.dma_start(bv_sb, bv16[:, :])
        nc.sync.dma_start(Wv_sb, Wv16[:, :, :])
        nc.sync.dma_start(vT_sb, vT16[:, :, :])
        nc.sync.dma_start(Wq_sb, Wq16[:, :, :])

        qT_tiles = {}

        def dma_qT(qb):
            t = qTp.tile([P, 8, IB], bf16, tag="qT", name=f"qT{qb}")
            nc.sync.dma_start(t, qT16[:, :, qb * IB : (qb + 1) * IB])
            qT_tiles[qb] = t

        dma_qT(0)
        dma_qT(1)
        nc.sync.dma_start(Wo_sb, Wo16[:, :, :])

        make_identity(nc, ident)
        nc.gpsimd.memset(ones1, 1.0)
        nc.gpsimd.memset(vh16[:, :, :, DK], 1.0)

        # ---------------- prologue projections (all bf16) ----------------
        for do in range(8):
            pk = ps_sc.tile([P, 2, IB], f32, tag="sc", name=f"kp{do}")
            for kb in range(2):
                for s in range(4):
                    nc.tensor.matmul(
                        pk[:, kb, :],
                        lhsT=Wk_sb[:, s, do * P : (do + 1) * P],
                        rhs=kT_sb[:, s, kb * IB : (kb + 1) * IB],
                        start=(s == 0), stop=(s == 3),
                    )
            nc.scalar.activation(
                kh16[:, do, :], pk.rearrange("p a b -> p (a b)"), Id,
                bias=bk_sb[:, do : do + 1], scale=1.0,
            )

        for jt in range(n_jt):
            pv_ = ps_sc.tile([P, 2, IB], f32, tag="sc", name=f"vp{jt}")
            for db in range(2):
                for s in range(4):
                    nc.tensor.matmul(
                        pv_[:, db, :],
                        lhsT=vT_sb[:, s, jt * P : (jt + 1) * P],
                        rhs=Wv_sb[:, s, db * IB : (db + 1) * IB],
                        start=(s == 0), stop=False,
                    )
                nc.tensor.matmul(
                    pv_[:, db, :], lhsT=ones1, rhs=bv_sb[:, db * IB : (db + 1) * IB],
                    start=False, stop=True,
                )
            for db in range(2):
                nc.scalar.activation(
                    vh16[:, jt, db * 8 : (db + 1) * 8, 0:DK],
                    pv_[:, db, :].rearrange("p (h d) -> p h d", d=DK),
                    Copy,
                )

        def make_qp_chunk(qb, do, qh_sb):
            def run():
                pq = ps_g.tile([P, IB], f32, tag="psg", name=f"qp{qb}_{do}")
                for s in range(8):
                    nc.tensor.matmul(
                        pq,
                        lhsT=Wq_sb[:, s, do * P : (do + 1) * P],
                        rhs=qT_tiles[qb][:, s, :],
                        start=(s == 0), stop=(s == 7),
                    )
                nc.scalar.activation(
                    qh_sb[:, do, :], pq, Id,
                    bias=bq_sb[:, do : do + 1], scale=1.0,
                )
            return run

        qh_tiles = {}
        qh_tiles[0] = qhp.tile([P, 8, IB], bf16, tag="qh", name="qh0")
        for do in range(8):
            make_qp_chunk(0, do, qh_tiles[0])()

        # ---------------- main loop ----------------
        exp_idx = [0]

        def emit_scores(qb, h, jp):
            base = (h % 2) * DK
            c = h // 2
            sct = ps_sc.tile([P, 2, IB], f32, tag="sc", name=f"s{qb}_{h}_{jp}")
            for u in range(2):
                jt = 2 * jp + u
                nc.tensor.matmul(
                    sct[:, u, :],
                    lhsT=kh16[base : base + DK, c, jt * P : (jt + 1) * P],
                    rhs=qh_tiles[qb][base : base + DK, c, :],
                    start=True, stop=True,
                )
            et = e_pool.tile([P, 2, IB], bf16, tag="e", name=f"e{qb}_{h}_{jp}")
            i = exp_idx[0]
            exp_idx[0] += 1
            if (i * ACT_EXP_NUM) % 32 < ACT_EXP_NUM:
                nc.scalar.activation(et, sct, Exp, scale=ESCALE)
            else:
                nc.vector.tensor_scalar(
                    et.bitcast(u16), sct, C1, KP, Alu.mult, Alu.add
                )
            return et

        # PV pipeline state for the PREVIOUS head-group
        def pv_chunks(qb, hg, e_tiles, ants):
            """Returns 16 closures: 4 qt x 4 h4 accumulation chunks; the last
            chunk of each qt appends recip+norm."""
            pvt_ref = [None]
            chunks = []
            for qt in range(4):
                for h4 in range(4):
                    def run(qt=qt, h4=h4):
                        if h4 == 0:
                            pvt_ref[0] = ps_pv.tile(
                                [P, 4, HW], f32, tag="pv", name=f"pv{qb}_{hg}_{qt}"
                            )
                        pvt = pvt_ref[0]
                        h = 4 * hg + h4
                        for jt in range(n_jt):
                            et = e_tiles[h4][jt // 2]
                            nc.tensor.matmul(
                                pvt[:, h4, :],
                                lhsT=et[:, jt % 2, qt * P : (qt + 1) * P],
                                rhs=vh16[:, jt, h, :],
                                start=(jt == 0), stop=(jt == n_jt - 1),
                            )
                        if h4 == 3:
                            rec = rec_pool.tile(
                                [P, 4], f32, tag="rec", name=f"rc{qb}_{hg}_{qt}"
                            )
                            nc.vector.reciprocal(rec, pvt[:, :, DK])
                            ant = an_pool.tile(
                                [P, 4, DK], bf16, tag="an", name=f"an{qb}_{hg}_{qt}"
                            )
                            nc.vector.tensor_tensor(
                                ant, pvt[:, :, 0:DK],
                                rec[:, :, None].to_broadcast([P, 4, DK]),
                                Alu.mult,
                            )
                            ants[hg][qt] = ant
                    chunks.append(run)
            return chunks

        def make_tr_chunk(qb, qt, ants, cat_sb):
            def run():
                trt = ps_tr.tile([P, 8, P], bf16, tag="tr", name=f"tr{qb}_{qt}")
                for g in range(8):
                    hg, l = g // 2, g % 2
                    nc.tensor.transpose(
                        trt[:, g, :], ants[hg][qt][:, 2 * l : 2 * l + 2, :], ident
                    )
                nc.vector.tensor_copy(
                    cat_sb[:, :, qt * P : (qt + 1) * P], trt
                )
            return run

        def make_op_chunk(qb, qt, db, cat_sb):
            def run():
                po = ps_g.tile([P, IB], f32, tag="psg", name=f"op{qb}_{qt}_{db}")
                for g in range(8):
                    nc.tensor.matmul(
                        po,
                        lhsT=cat_sb[:, g, qt * P : (qt + 1) * P],
                        rhs=Wo_sb[:, g, db * IB : (db + 1) * IB],
                        start=(g == 0), stop=(g == 7),
                    )
                ot = osb_pool.tile([P, IB], bf16, tag="osb", name=f"o{qb}_{qt}_{db}")
                nc.vector.tensor_copy(ot, po)
                nc.sync.dma_start(
                    out16[
                        qb * IB + qt * P : qb * IB + (qt + 1) * P,
                        db * IB : (db + 1) * IB,
                    ],
                    ot,
                )
            return run

        extras = []
        ei = [0]

        def pop_extra(n=1):
            m = 0
            while m < n and ei[0] < len(extras):
                extras[ei[0]]()
                ei[0] += 1
                m += 1

        # carry: pv chunks of the previous head-group
        carry = []   # list of closures still to emit
        all_ants = {}

        for qb in range(n_ib):
            cat_sb = cat_pool.tile([P, 8, IB], bf16, tag="cat", name=f"cat{qb}")
            ants = [[None] * 4 for _ in range(4)]
            all_ants[qb] = ants
            if qb == 0:
                qh_tiles[1] = qhp.tile([P, 8, IB], bf16, tag="qh", name="qh1")
                extras = [make_qp_chunk(1, do, qh_tiles[1]) for do in range(8)]
                ei = [0]
            for hg in range(4):
                e_tiles = [[None] * 4 for _ in range(4)]
                for h4 in range(4):
                    h = 4 * hg + h4
                    for jp in range(4):
                        e_tiles[h4][jp] = emit_scores(qb, h, jp)
                        # interleave: one pv chunk of prev head-group + extra
                        if carry:
                            carry.pop(0)()
                        if hg > 0 or qb == 0:
                            # hg0 of later blocks: carry (pv of prev block's
                            # hg3) must fully drain before tr extras run
                            pop_extra(1)
                carry_new = pv_chunks(qb, hg, e_tiles, ants)
                if hg == 3 and qb == n_ib - 1:
                    # last head-group of last block: interleave PV flush with
                    # per-qt transposes and out-projection (pipelined tail)
                    for qt in range(4):
                        for i in range(4):
                            carry_new[qt * 4 + i]()
                        make_tr_chunk(qb, qt, ants, cat_sb)()
                        for db in range(2):
                            make_op_chunk(qb, qt, db, cat_sb)()
                    carry = []
                    tail_done = True
                else:
                    # flush any leftover of previous carry, then swap
                    for c in carry:
                        c()
                    carry = carry_new

            # assemble next block's extras (run during next block)
            nx = []
            # tr chunks for THIS block can only run after its last hg PV done
            # -> schedule this block's tr+op in the NEXT block's extras
            if qb + 1 < n_ib:
                qh_tiles[qb + 1 + 1] = (
                    qhp.tile([P, 8, IB], bf16, tag="qh", name=f"qh{qb+2}")
                    if qb + 2 < n_ib else None
                )
                dma_next = qb + 2
                if dma_next < n_ib:
                    dma_qT(dma_next)
                for qt in range(4):
                    nx.append(make_tr_chunk(qb, qt, ants, cat_sb))
                if qb + 2 < n_ib:
                    for do in range(8):
                        nx.append(make_qp_chunk(qb + 2, do, qh_tiles[qb + 2]))
                for qt in range(4):
                    for db in range(2):
                        nx.append(make_op_chunk(qb, qt, db, cat_sb))
                pop_extra(len(extras))  # safety flush of old extras
                extras = nx
                ei = [0]
            else:
                pop_extra(len(extras))

    nc.compile()
    return nc


def _get_nc():
    if "nc" not in _cache:
        _cache["nc"] = _build_nc()
    return _cache["nc"]


def kernel(**inputs):
    import ml_dtypes

    from concourse.bass_utils import run_bass_kernel_spmd

    bf16 = ml_dtypes.bfloat16
    f32 = np.float32

    nc = _get_nc()

    Wq = np.asarray(inputs["Wq"], f32)
    Wk = np.asarray(inputs["Wk"], f32)
    Wv = np.asarray(inputs["Wv"], f32)
    Wo = np.asarray(inputs["Wo"], f32)
    bq = np.asarray(inputs["bq"], f32)
    bk = np.asarray(inputs["bk"], f32)
    bv = np.asarray(inputs["bv"], f32)
    bo = np.asarray(inputs["bo"], f32)

    shared = {
        "Wq16": np.ascontiguousarray(
            Wq.reshape(8, P, DM).transpose(1, 0, 2).astype(bf16)),
        "Wk16": np.ascontiguousarray(
            Wk.reshape(4, P, DM).transpose(1, 0, 2).astype(bf16)),
        "Wv16": np.ascontiguousarray(
            Wv.reshape(4, P, DM).transpose(1, 0, 2).astype(bf16)),
        "Wo16": np.ascontiguousarray(
            Wo.reshape(8, P, DM).transpose(1, 0, 2).astype(bf16)),
        "bqp": np.ascontiguousarray(bq.reshape(8, P).T),
        "bkp": np.ascontiguousarray(bk.reshape(8, P).T),
        "bv16": np.ascontiguousarray(bv.reshape(1, DM).astype(bf16)),
    }

    q = np.asarray(inputs["q"], f32)
    k = np.asarray(inputs["k"], f32)
    v = np.asarray(inputs["v"], f32)

    in_maps = []
    for b in range(B):
        m = dict(shared)
        m["qT16"] = np.ascontiguousarray(
            q[b].T.reshape(8, P, LQ).transpose(1, 0, 2).astype(bf16))
        m["kT16"] = np.ascontiguousarray(
            k[b].T.reshape(4, P, LKV).transpose(1, 0, 2).astype(bf16))
        m["vT16"] = np.ascontiguousarray(
            v[b].T.reshape(4, P, LKV).transpose(1, 0, 2).astype(bf16))
        in_maps.append(m)

    res = run_bass_kernel_spmd(nc, in_maps, list(range(B)), trace=TRACE)
    global LAST_RESULT
    LAST_RESULT = res
    out = np.stack(
        [np.asarray(res.results[b]["out16"]).astype(f32) for b in range(B)], axis=0
    )
    return out + bo[None, None, :]
